# revision 27
# baseline (speedup 1.0000x reference)
"""Trainium2 Bass kernel for a scalar-input GRU (B=512, T=128, H=512) + ReLU/Linear head.

Strategy: data-parallel over batch across 8 NeuronCores (64 rows each).
Per core, per time step:
  - PSUM accumulates the full pre-activations gh = h @ w_hh.T + gx + biases via
    bf16 matmuls (fp32 PSUM accumulate): a K=2 "augmented" chunk (rows = [x_t; 1])
    folds x_t*w_ih + bias into the same accumulation group as the 4 K=128 h-chunks.
  - ACT applies sigmoid/tanh; DVE does the gate algebra, column-chunked so the
    tail pipelines into the next step's matmuls.
  - PE transposes h_new back into the [H-chunk, B] stationary layout.
The T=128 recurrence is fully unrolled (no hardware loop back-edges).
"""

import sys

sys.path.insert(0, "/opt/trn_rl_repo")

import numpy as np

import concourse.bacc as bacc
import concourse.bass as bass
import concourse.mybir as mybir
import concourse.tile as tile
from concourse.bass_utils import run_bass_kernel_spmd
from concourse.masks import make_identity

N_CORES = 8
B_FULL, T_FULL, H = 512, 128, 512
B = B_FULL // N_CORES  # 64 batch rows per core
G3 = 3 * H  # 1536
NK = H // 128  # 4 contraction chunks
HC = H // 2  # 256-wide tail chunks
F32 = mybir.dt.float32
BF16 = mybir.dt.bfloat16
AF = mybir.ActivationFunctionType


def build_nc(T: int = T_FULL) -> bass.Bass:
    nc = bacc.Bacc("TRN2", target_bir_lowering=False, debug=False)

    x_d = nc.dram_tensor("x", [B, T], F32, kind="ExternalInput")
    whh_d = nc.dram_tensor("w_hh", [G3, H], F32, kind="ExternalInput")
    wih_d = nc.dram_tensor("w_ih", [G3, 1], F32, kind="ExternalInput")
    bih_d = nc.dram_tensor("b_ih", [G3], F32, kind="ExternalInput")
    bhh_d = nc.dram_tensor("b_hh", [G3], F32, kind="ExternalInput")
    fcw_d = nc.dram_tensor("fc_w", [1, H], F32, kind="ExternalInput")
    fcb_d = nc.dram_tensor("fc_b", [1], F32, kind="ExternalInput")
    out_d = nc.dram_tensor("out", [B, 1], F32, kind="ExternalOutput")

    with tile.TileContext(nc) as tc:
        _body(tc, T, x_d, whh_d, wih_d, bih_d, bhh_d, fcw_d, fcb_d, out_d)
    nc.compile()
    return nc


def _body(tc, T, x_d, whh_d, wih_d, bih_d, bhh_d, fcw_d, fcb_d, out_d):
    nc = tc.nc
    with (
        tc.tile_pool(name="const", bufs=1) as cpool,
        tc.tile_pool(name="state", bufs=2) as spool,
        tc.tile_pool(name="work", bufs=3) as wpool,
        tc.tile_pool(name="psgh", bufs=2, space="PSUM") as ppool,
        tc.tile_pool(name="pstp", bufs=1, space="PSUM") as tpool,
        tc.tile_pool(name="psgx", bufs=1, space="PSUM") as gpool,
    ):
        # ---- one-time prep ----
        ident64 = cpool.tile([64, 64], F32)
        make_identity(nc, ident64)
        ident128 = cpool.tile([128, 128], F32)
        make_identity(nc, ident128)

        # augmented stationary source: row0 = x in (t, b) order, row1 = ones
        # bf16 pair-split of x on 64 partitions, then gather to (t, b) rows
        x_sb = cpool.tile([B, T], F32)
        nc.sync.dma_start(out=x_sb[:, :], in_=x_d[:, :])
        xhi_b = cpool.tile([B, T], BF16)
        nc.vector.tensor_copy(xhi_b[:, :], x_sb[:, :])
        xhi_f = cpool.tile([B, T], F32)
        nc.vector.tensor_copy(xhi_f[:, :], xhi_b[:, :])
        xlo_f = cpool.tile([B, T], F32)
        nc.vector.tensor_sub(xlo_f[:, :], x_sb[:, :], xhi_f[:, :])
        xlo_b = cpool.tile([B, T], BF16)
        nc.vector.tensor_copy(xlo_b[:, :], xlo_f[:, :])
        xhi_d = nc.dram_tensor("xhi_scratch", [B, T], BF16, kind="Internal")
        xlo_d = nc.dram_tensor("xlo_scratch", [B, T], BF16, kind="Internal")
        nc.sync.dma_start(out=xhi_d[:, :], in_=xhi_b[:, :])
        nc.sync.dma_start(out=xlo_d[:, :], in_=xlo_b[:, :])
        xaug = cpool.tile([5, T * B], BF16)
        nc.gpsimd.memset(xaug[:, :], 1.0)
        for (row, srcd) in ((0, xhi_d), (1, xlo_d), (2, xhi_d)):
            nc.sync.dma_start(
                out=xaug[row : row + 1, :].rearrange("p (t b) -> p t b", t=T),
                in_=srcd[:, :].transpose([1, 0]).unsqueeze(0),
            )

        # w_hh.T chunks: wT[p, k*G3 + j] = w_hh[j, 128k + p]  (bf16)
        wstage = cpool.tile([128, (G3 // 128) * H], F32)
        nc.sync.dma_start(
            out=wstage[:, :].rearrange("p (c h) -> p c h", h=H),
            in_=whh_d[:, :].rearrange("(c p) h -> c p h", p=128).transpose([1, 0, 2]),
        )
        wT = cpool.tile([128, NK * G3], BF16)
        for c in range(G3 // 128):
            for k in range(NK):
                tp = ppool.tile([128, 128], F32, tag="ghn", name=f"wprep_{c}_{k}")
                nc.tensor.transpose(
                    tp[:, :], wstage[:, c * H + k * 128 : c * H + (k + 1) * 128], ident128
                )
                nc.vector.tensor_copy(
                    wT[:, k * G3 + c * 128 : k * G3 + (c + 1) * 128], tp[:, :]
                )

        # staged fp32 rows on partition 0
        wi_f = cpool.tile([1, G3], F32)
        nc.sync.dma_start(out=wi_f[:, :], in_=wih_d[:, :].rearrange("g one -> (g one)")[None, :])
        bsum = cpool.tile([1, G3], F32)
        nc.sync.dma_start(out=bsum[:, :], in_=bhh_d[None, :])
        bihs = cpool.tile([1, G3], F32)
        nc.sync.dma_start(out=bihs[:, :], in_=bih_d[None, :])
        nc.vector.tensor_add(bsum[:, 0 : 2 * H], bsum[:, 0 : 2 * H], bihs[:, 0 : 2 * H])
        nc.gpsimd.memset(wi_f[:, 2 * H : G3], 0.0)  # n gate: x-path excluded

        def pair_split(name, srcrow):
            hi_b = cpool.tile([1, G3], BF16, name=f"{name}_hi_b")
            nc.vector.tensor_copy(hi_b[:, :], srcrow)
            hi_f = cpool.tile([1, G3], F32, name=f"{name}_hi_f")
            nc.vector.tensor_copy(hi_f[:, :], hi_b[:, :])
            lo_f = cpool.tile([1, G3], F32, name=f"{name}_lo_f")
            nc.vector.tensor_sub(lo_f[:, :], srcrow, hi_f[:, :])
            lo_b = cpool.tile([1, G3], BF16, name=f"{name}_lo_b")
            nc.vector.tensor_copy(lo_b[:, :], lo_f[:, :])
            return hi_b, lo_b

        wi_hi, wi_lo = pair_split("wi", wi_f[:, :])
        b_hi, b_lo = pair_split("bsum", bsum[:, :])
        bih_hi, bih_lo = pair_split("bih", bihs[:, :])

        AUG = cpool.tile([5, G3], BF16)
        nc.sync.dma_start(out=AUG[0:1, :], in_=wi_hi[:, :])
        nc.sync.dma_start(out=AUG[1:2, :], in_=wi_hi[:, :])
        nc.sync.dma_start(out=AUG[2:3, :], in_=wi_lo[:, :])
        nc.sync.dma_start(out=AUG[3:4, :], in_=b_hi[:, :])
        nc.sync.dma_start(out=AUG[4:5, :], in_=b_lo[:, :])
        # n-gate x-path (added outside the r* product): wi_n and b_ih_n pairs
        wiN_f = cpool.tile([1, H], F32)
        nc.sync.dma_start(
            out=wiN_f[:, :], in_=wih_d[2 * H : G3, :].rearrange("g one -> (g one)")[None, :]
        )
        wiN_hi_b = cpool.tile([1, H], BF16)
        nc.vector.tensor_copy(wiN_hi_b[:, :], wiN_f[:, :])
        wiN_hi_f = cpool.tile([1, H], F32)
        nc.vector.tensor_copy(wiN_hi_f[:, :], wiN_hi_b[:, :])
        wiN_lo_f = cpool.tile([1, H], F32)
        nc.vector.tensor_sub(wiN_lo_f[:, :], wiN_f[:, :], wiN_hi_f[:, :])
        wiN_lo_b = cpool.tile([1, H], BF16)
        nc.vector.tensor_copy(wiN_lo_b[:, :], wiN_lo_f[:, :])
        AUGN = cpool.tile([5, H], BF16)
        nc.sync.dma_start(out=AUGN[0:1, :], in_=wiN_hi_b[:, :])
        nc.sync.dma_start(out=AUGN[1:2, :], in_=wiN_hi_b[:, :])
        nc.sync.dma_start(out=AUGN[2:3, :], in_=wiN_lo_b[:, :])
        nc.sync.dma_start(out=AUGN[3:4, :], in_=bih_hi[:, 2 * H : G3])
        nc.sync.dma_start(out=AUGN[4:5, :], in_=bih_lo[:, 2 * H : G3])

        # fc weights: fcw[p, k] = fc_w[0, 128k + p]
        fcwf = cpool.tile([128, NK], F32)
        nc.sync.dma_start(
            out=fcwf[:, :],
            in_=fcw_d[:, :]
            .rearrange("one (k p) -> one k p", p=128)
            .transpose([2, 0, 1])
            .rearrange("p one k -> p (one k)"),
        )
        fcbf = cpool.tile([1, 1], F32)
        nc.sync.dma_start(out=fcbf[:, :], in_=fcb_d[None, :])
        onesf = cpool.tile([1, B], F32)
        nc.gpsimd.memset(onesf[:, :], 1.0)

        # state init: h = 0 (fp32 master, 256-col chunks) + bf16 transposed layout
        ha = spool.tile([B, HC], F32, tag="ha", name="ha_init")
        hb = spool.tile([B, HC], F32, tag="hb", name="hb_init")
        nc.gpsimd.memset(ha[:, :], 0.0)
        nc.gpsimd.memset(hb[:, :], 0.0)
        hTa = spool.tile([128, 2 * B], BF16, tag="hTa", name="hTa_init")
        hTb = spool.tile([128, 2 * B], BF16, tag="hTb", name="hTb_init")
        nc.gpsimd.memset(hTa[:, :], 0.0)
        nc.gpsimd.memset(hTb[:, :], 0.0)

        # ---- the recurrence, fully unrolled ----
        for t in range(T):
            psR = ppool.tile([B, 512], F32, tag="ghr", name=f"psR_{t}", bufs=2)
            psZ = ppool.tile([B, 512], F32, tag="ghz", name=f"psZ_{t}", bufs=2)
            psN = ppool.tile([B, 512], F32, tag="ghn", name=f"psN_{t}", bufs=2)
            pst = (psR, psZ, psN)
            psg = gpool.tile([B, H], F32, tag="gx", name=f"psg_{t}")
            xs = xaug[0:5, t * B : (t + 1) * B]  # [5, 64]: [x_hi; x_lo; x_hi; 1; 1]
            hT_ = (hTa, hTa, hTb, hTb)
            hoff = (0, B, 0, B)

            def hmm(g, k):
                nc.tensor.matmul(
                    pst[g][:, :],
                    hT_[k][:, hoff[k] : hoff[k] + B],
                    wT[:, k * G3 + g * 512 : k * G3 + (g + 1) * 512],
                    start=False,
                    stop=(k == NK - 1),
                )

            # aug matmuls first (hoistable into the previous step's tail),
            # then k-half-major h-matmuls: everything needing only hTa before
            # anything needing hTb, gates ordered r, n, z within each half.
            nc.tensor.matmul(psR[:, :], xs, AUG[0:5, 0:512], start=True, stop=False)
            nc.tensor.matmul(
                psN[:, :], xs, AUG[0:5, 1024:1536], start=True, stop=False
            )
            nc.tensor.matmul(
                psZ[:, :], xs, AUG[0:5, 512:1024], start=True, stop=False
            )
            nc.tensor.matmul(psg[:, :], xs, AUGN[0:5, :], start=True, stop=True)
            for g in (0, 2):
                for k in (0, 1):
                    hmm(g, k)
            for g in (0, 2):
                for k in (2, 3):
                    hmm(g, k)
            for k in range(NK):
                hmm(1, k)

            # r = sigmoid(pre_r)
            r = wpool.tile([B, H], F32, tag="r", name=f"r_{t}")
            nc.scalar.activation(r[:, :], psR[:, :], AF.Sigmoid)

            # n = tanh(gx_n + r * pre_n), 256-col chunks
            n = wpool.tile([B, H], F32, tag="n", name=f"n_{t}")
            for c in range(2):
                cs = slice(c * HC, (c + 1) * HC)
                m = wpool.tile([B, HC], F32, tag=f"m{c}", name=f"m{c}_{t}")
                nc.vector.tensor_mul(m[:, :], r[:, cs], psN[:, cs])
                m2 = wpool.tile([B, HC], F32, tag=f"m2{c}", name=f"m2{c}_{t}")
                nc.vector.tensor_add(m2[:, :], m[:, :], psg[:, cs])
                nc.scalar.activation(n[:, cs], m2[:, :], AF.Tanh)

            # z = sigmoid(pre_z)
            z = wpool.tile([B, H], F32, tag="z", name=f"z_{t}")
            nc.scalar.activation(z[:, :], psZ[:, :], AF.Sigmoid)
            # h_new = n + z*(h - n)
            h_old = (ha, hb)
            new_h, new_hT = [], []
            tp_full = tpool.tile([128, NK * B], F32, tag="tp", name=f"tp_{t}", bufs=1)
            for c in range(2):
                tp = tp_full[:, 2 * c * B : 2 * (c + 1) * B]
                cs = slice(c * HC, (c + 1) * HC)
                d = wpool.tile([B, HC], F32, tag=f"d{c}", name=f"d{c}_{t}")
                nc.vector.tensor_sub(d[:, :], h_old[c][:, :], n[:, cs])
                e = wpool.tile([B, HC], F32, tag=f"e{c}", name=f"e{c}_{t}")
                nc.vector.tensor_mul(e[:, :], z[:, cs], d[:, :])
                hn = spool.tile(
                    [B, HC], F32, tag=("ha", "hb")[c], name=f"h{('a', 'b')[c]}_{t}"
                )
                nc.vector.tensor_add(hn[:, :], n[:, cs], e[:, :])
                for kk in range(2):
                    nc.tensor.transpose(
                        tp[:, kk * B : (kk + 1) * B],
                        hn[:, kk * 128 : (kk + 1) * 128],
                        ident64,
                    )
                hTn = spool.tile(
                    [128, 2 * B],
                    BF16,
                    tag=("hTa", "hTb")[c],
                    name=f"hT{('a', 'b')[c]}_{t}",
                )
                nc.vector.tensor_copy(hTn[:, :], tp[:, :])
                new_h.append(hn)
                new_hT.append(hTn)

            ha, hb = new_h
            hTa, hTb = new_hT

        # ---- head: out = relu(h) @ fc_w.T + fc_b ----
        reluh = wpool.tile([B, H], F32, tag="r", name="reluh")
        nc.scalar.activation(reluh[:, 0:HC], ha[:, :], AF.Relu)
        nc.scalar.activation(reluh[:, HC:H], hb[:, :], AF.Relu)
        tpf = tpool.tile([128, NK * B], F32, tag="tp", name="tp_fc", bufs=1)
        for k in range(NK):
            nc.tensor.transpose(
                tpf[:, k * B : (k + 1) * B], reluh[:, k * 128 : (k + 1) * 128], ident64
            )
        rhT = spool.tile([128, NK * B], F32, tag="rhT", name="rhT")
        nc.vector.tensor_copy(rhT[:, :], tpf[:, :])

        psf = gpool.tile([B, H], F32, tag="gx", name="ps_fc")
        nc.tensor.matmul(psf[:, 0:1], onesf[:, :], fcbf[0:1, 0:1], start=True, stop=False)
        for k in range(NK):
            nc.tensor.matmul(
                psf[:, 0:1],
                rhT[:, k * B : (k + 1) * B],
                fcwf[:, k : k + 1],
                start=False,
                stop=(k == NK - 1),
            )
        outsb = wpool.tile([B, 1], F32, tag="outsb", name="out_sb")
        nc.vector.tensor_copy(outsb[:, :], psf[:, 0:1])
        nc.sync.dma_start(out=out_d[:, :], in_=outsb[:, :])


_NC_CACHE: dict[int, bass.Bass] = {}


def _get_nc(T: int = T_FULL) -> bass.Bass:
    if T not in _NC_CACHE:
        _NC_CACHE[T] = build_nc(T)
    return _NC_CACHE[T]


def kernel(x, w_ih, w_hh, b_ih, b_hh, fc_w, fc_b, _trace=False, _tmpdir=None):
    x = np.ascontiguousarray(np.asarray(x, dtype=np.float32))
    nc = _get_nc(x.shape[1])
    shared = {
        "w_hh": np.ascontiguousarray(np.asarray(w_hh, np.float32)),
        "w_ih": np.ascontiguousarray(np.asarray(w_ih, np.float32)),
        "b_ih": np.ascontiguousarray(np.asarray(b_ih, np.float32)),
        "b_hh": np.ascontiguousarray(np.asarray(b_hh, np.float32)),
        "fc_w": np.ascontiguousarray(np.asarray(fc_w, np.float32)),
        "fc_b": np.ascontiguousarray(np.asarray(fc_b, np.float32)),
    }
    in_maps = [{"x": x[c * B : (c + 1) * B], **shared} for c in range(N_CORES)]
    res = run_bass_kernel_spmd(
        nc, in_maps, list(range(N_CORES)), trace=_trace, tmpdir=_tmpdir
    )
    out = np.concatenate([res.results[c]["out"] for c in range(N_CORES)], axis=0)
    if _trace:
        return out, res
    return out


# revision 30
# speedup vs baseline: 1.0413x; 1.0413x over previous
"""Trainium2 Bass kernel for a scalar-input GRU (B=512, T=128, H=512) + ReLU/Linear head.

Strategy: data-parallel over batch across 8 NeuronCores (64 rows each).
Per core, per time step:
  - PSUM accumulates the full pre-activations gh = h @ w_hh.T + gx + biases via
    bf16 matmuls (fp32 PSUM accumulate): a K=2 "augmented" chunk (rows = [x_t; 1])
    folds x_t*w_ih + bias into the same accumulation group as the 4 K=128 h-chunks.
  - ACT applies sigmoid/tanh; DVE does the gate algebra, column-chunked so the
    tail pipelines into the next step's matmuls.
  - PE transposes h_new back into the [H-chunk, B] stationary layout.
The T=128 recurrence is fully unrolled (no hardware loop back-edges).
"""

import sys

sys.path.insert(0, "/opt/trn_rl_repo")

import numpy as np

import concourse.bacc as bacc
import concourse.bass as bass
import concourse.mybir as mybir
import concourse.tile as tile
from concourse.bass_utils import run_bass_kernel_spmd
from concourse.masks import make_identity

N_CORES = 8
B_FULL, T_FULL, H = 512, 128, 512
B = B_FULL // N_CORES  # 64 batch rows per core
G3 = 3 * H  # 1536
NK = H // 128  # 4 contraction chunks
HC = H // 2  # 256-wide tail chunks
F32 = mybir.dt.float32
BF16 = mybir.dt.bfloat16
AF = mybir.ActivationFunctionType


def build_nc(T: int = T_FULL) -> bass.Bass:
    nc = bacc.Bacc("TRN2", target_bir_lowering=False, debug=False)

    x_d = nc.dram_tensor("x", [B, T], F32, kind="ExternalInput")
    whh_d = nc.dram_tensor("w_hh", [G3, H], F32, kind="ExternalInput")
    wih_d = nc.dram_tensor("w_ih", [G3, 1], F32, kind="ExternalInput")
    bih_d = nc.dram_tensor("b_ih", [G3], F32, kind="ExternalInput")
    bhh_d = nc.dram_tensor("b_hh", [G3], F32, kind="ExternalInput")
    fcw_d = nc.dram_tensor("fc_w", [1, H], F32, kind="ExternalInput")
    fcb_d = nc.dram_tensor("fc_b", [1], F32, kind="ExternalInput")
    out_d = nc.dram_tensor("out", [B, 1], F32, kind="ExternalOutput")

    with tile.TileContext(nc) as tc:
        _body(tc, T, x_d, whh_d, wih_d, bih_d, bhh_d, fcw_d, fcb_d, out_d)
    nc.compile()
    return nc


def _body(tc, T, x_d, whh_d, wih_d, bih_d, bhh_d, fcw_d, fcb_d, out_d):
    nc = tc.nc
    with (
        tc.tile_pool(name="const", bufs=1) as cpool,
        tc.tile_pool(name="state", bufs=2) as spool,
        tc.tile_pool(name="work", bufs=3) as wpool,
        tc.tile_pool(name="psgh", bufs=2, space="PSUM") as ppool,
        tc.tile_pool(name="pstp", bufs=1, space="PSUM") as tpool,
        tc.tile_pool(name="psgx", bufs=1, space="PSUM") as gpool,
    ):
        # ---- one-time prep ----
        ident64 = cpool.tile([64, 64], F32)
        make_identity(nc, ident64)
        ident128 = cpool.tile([128, 128], F32)
        make_identity(nc, ident128)

        # augmented stationary source: row0 = x in (t, b) order, row1 = ones
        # bf16 pair-split of x on 64 partitions, then gather to (t, b) rows
        x_sb = cpool.tile([B, T], F32)
        nc.sync.dma_start(out=x_sb[:, :], in_=x_d[:, :])
        xhi_b = cpool.tile([B, T], BF16)
        nc.vector.tensor_copy(xhi_b[:, :], x_sb[:, :])
        xhi_f = cpool.tile([B, T], F32)
        nc.vector.tensor_copy(xhi_f[:, :], xhi_b[:, :])
        xlo_f = cpool.tile([B, T], F32)
        nc.vector.tensor_sub(xlo_f[:, :], x_sb[:, :], xhi_f[:, :])
        xlo_b = cpool.tile([B, T], BF16)
        nc.vector.tensor_copy(xlo_b[:, :], xlo_f[:, :])
        xhi_d = nc.dram_tensor("xhi_scratch", [B, T], BF16, kind="Internal")
        xlo_d = nc.dram_tensor("xlo_scratch", [B, T], BF16, kind="Internal")
        nc.sync.dma_start(out=xhi_d[:, :], in_=xhi_b[:, :])
        nc.sync.dma_start(out=xlo_d[:, :], in_=xlo_b[:, :])
        xaug = cpool.tile([5, T * B], BF16)
        nc.gpsimd.memset(xaug[:, :], 1.0)
        for (row, srcd) in ((0, xhi_d), (1, xlo_d), (2, xhi_d)):
            nc.sync.dma_start(
                out=xaug[row : row + 1, :].rearrange("p (t b) -> p t b", t=T),
                in_=srcd[:, :].transpose([1, 0]).unsqueeze(0),
            )

        # w_hh.T chunks: wT[p, k*G3 + j] = w_hh[j, 128k + p]  (bf16)
        wstage = cpool.tile([128, (G3 // 128) * H], F32)
        nc.sync.dma_start(
            out=wstage[:, :].rearrange("p (c h) -> p c h", h=H),
            in_=whh_d[:, :].rearrange("(c p) h -> c p h", p=128).transpose([1, 0, 2]),
        )
        wT = cpool.tile([128, NK * G3], BF16)
        for c in range(G3 // 128):
            for k in range(NK):
                tp = ppool.tile([128, 128], F32, tag="ghn", name=f"wprep_{c}_{k}")
                nc.tensor.transpose(
                    tp[:, :], wstage[:, c * H + k * 128 : c * H + (k + 1) * 128], ident128
                )
                nc.vector.tensor_copy(
                    wT[:, k * G3 + c * 128 : k * G3 + (c + 1) * 128], tp[:, :]
                )

        # staged fp32 rows on partition 0
        wi_f = cpool.tile([1, G3], F32)
        nc.sync.dma_start(out=wi_f[:, :], in_=wih_d[:, :].rearrange("g one -> (g one)")[None, :])
        bsum = cpool.tile([1, G3], F32)
        nc.sync.dma_start(out=bsum[:, :], in_=bhh_d[None, :])
        bihs = cpool.tile([1, G3], F32)
        nc.sync.dma_start(out=bihs[:, :], in_=bih_d[None, :])
        nc.vector.tensor_add(bsum[:, 0 : 2 * H], bsum[:, 0 : 2 * H], bihs[:, 0 : 2 * H])
        nc.gpsimd.memset(wi_f[:, 2 * H : G3], 0.0)  # n gate: x-path excluded

        def pair_split(name, srcrow):
            hi_b = cpool.tile([1, G3], BF16, name=f"{name}_hi_b")
            nc.vector.tensor_copy(hi_b[:, :], srcrow)
            hi_f = cpool.tile([1, G3], F32, name=f"{name}_hi_f")
            nc.vector.tensor_copy(hi_f[:, :], hi_b[:, :])
            lo_f = cpool.tile([1, G3], F32, name=f"{name}_lo_f")
            nc.vector.tensor_sub(lo_f[:, :], srcrow, hi_f[:, :])
            lo_b = cpool.tile([1, G3], BF16, name=f"{name}_lo_b")
            nc.vector.tensor_copy(lo_b[:, :], lo_f[:, :])
            return hi_b, lo_b

        wi_hi, wi_lo = pair_split("wi", wi_f[:, :])
        b_hi, b_lo = pair_split("bsum", bsum[:, :])
        bih_hi, bih_lo = pair_split("bih", bihs[:, :])

        AUG = cpool.tile([5, G3], BF16)
        nc.sync.dma_start(out=AUG[0:1, :], in_=wi_hi[:, :])
        nc.sync.dma_start(out=AUG[1:2, :], in_=wi_hi[:, :])
        nc.sync.dma_start(out=AUG[2:3, :], in_=wi_lo[:, :])
        nc.sync.dma_start(out=AUG[3:4, :], in_=b_hi[:, :])
        nc.sync.dma_start(out=AUG[4:5, :], in_=b_lo[:, :])
        # n-gate x-path (added outside the r* product): wi_n and b_ih_n pairs
        wiN_f = cpool.tile([1, H], F32)
        nc.sync.dma_start(
            out=wiN_f[:, :], in_=wih_d[2 * H : G3, :].rearrange("g one -> (g one)")[None, :]
        )
        wiN_hi_b = cpool.tile([1, H], BF16)
        nc.vector.tensor_copy(wiN_hi_b[:, :], wiN_f[:, :])
        wiN_hi_f = cpool.tile([1, H], F32)
        nc.vector.tensor_copy(wiN_hi_f[:, :], wiN_hi_b[:, :])
        wiN_lo_f = cpool.tile([1, H], F32)
        nc.vector.tensor_sub(wiN_lo_f[:, :], wiN_f[:, :], wiN_hi_f[:, :])
        wiN_lo_b = cpool.tile([1, H], BF16)
        nc.vector.tensor_copy(wiN_lo_b[:, :], wiN_lo_f[:, :])
        AUGN = cpool.tile([5, H], BF16)
        nc.sync.dma_start(out=AUGN[0:1, :], in_=wiN_hi_b[:, :])
        nc.sync.dma_start(out=AUGN[1:2, :], in_=wiN_hi_b[:, :])
        nc.sync.dma_start(out=AUGN[2:3, :], in_=wiN_lo_b[:, :])
        nc.sync.dma_start(out=AUGN[3:4, :], in_=bih_hi[:, 2 * H : G3])
        nc.sync.dma_start(out=AUGN[4:5, :], in_=bih_lo[:, 2 * H : G3])

        # fc weights: fcw[p, k] = fc_w[0, 128k + p]
        fcwf = cpool.tile([128, NK], F32)
        nc.sync.dma_start(
            out=fcwf[:, :],
            in_=fcw_d[:, :]
            .rearrange("one (k p) -> one k p", p=128)
            .transpose([2, 0, 1])
            .rearrange("p one k -> p (one k)"),
        )
        fcbf = cpool.tile([1, 1], F32)
        nc.sync.dma_start(out=fcbf[:, :], in_=fcb_d[None, :])
        onesf = cpool.tile([1, B], F32)
        nc.gpsimd.memset(onesf[:, :], 1.0)

        # state init: h = 0 (fp32 master, 256-col chunks) + bf16 transposed layout
        ha = spool.tile([B, HC], F32, tag="ha", name="ha_init")
        hb = spool.tile([B, HC], F32, tag="hb", name="hb_init")
        nc.gpsimd.memset(ha[:, :], 0.0)
        nc.gpsimd.memset(hb[:, :], 0.0)
        hTa = spool.tile([128, 2 * B], BF16, tag="hTa", name="hTa_init")
        hTb = spool.tile([128, 2 * B], BF16, tag="hTb", name="hTb_init")
        nc.gpsimd.memset(hTa[:, :], 0.0)
        nc.gpsimd.memset(hTb[:, :], 0.0)

        # ---- the recurrence, fully unrolled ----
        for t in range(T):
            psR = ppool.tile([B, 512], F32, tag="ghr", name=f"psR_{t}", bufs=2)
            psZ = ppool.tile([B, 512], F32, tag="ghz", name=f"psZ_{t}", bufs=2)
            psN = ppool.tile([B, 512], F32, tag="ghn", name=f"psN_{t}", bufs=2)
            pst = (psR, psZ, psN)
            psg = gpool.tile([B, H], F32, tag="gx", name=f"psg_{t}")
            xs = xaug[0:5, t * B : (t + 1) * B]  # [5, 64]: [x_hi; x_lo; x_hi; 1; 1]
            hT_ = (hTa, hTa, hTb, hTb)
            hoff = (0, B, 0, B)

            def hmm(g, k):
                nc.tensor.matmul(
                    pst[g][:, :],
                    hT_[k][:, hoff[k] : hoff[k] + B],
                    wT[:, k * G3 + g * 512 : k * G3 + (g + 1) * 512],
                    start=False,
                    stop=(k == NK - 1),
                )

            # aug matmuls first (hoistable into the previous step's tail),
            # then k-half-major h-matmuls: everything needing only hTa before
            # anything needing hTb, gates ordered r, n, z within each half.
            nc.tensor.matmul(psR[:, :], xs, AUG[0:5, 0:512], start=True, stop=False)
            nc.tensor.matmul(
                psN[:, :], xs, AUG[0:5, 1024:1536], start=True, stop=False
            )
            nc.tensor.matmul(
                psZ[:, :], xs, AUG[0:5, 512:1024], start=True, stop=False
            )
            nc.tensor.matmul(psg[:, :], xs, AUGN[0:5, :], start=True, stop=True)
            for g in (0, 2):
                for k in (0, 1):
                    hmm(g, k)
            for g in (0, 2):
                for k in (2, 3):
                    hmm(g, k)
            for k in range(NK):
                hmm(1, k)

            # r = sigmoid(pre_r), chunked so m0 only waits the first half
            r0 = wpool.tile([B, HC], F32, tag="r0", name=f"r0_{t}")
            nc.scalar.activation(r0[:, :], psR[:, 0:HC], AF.Sigmoid)
            r1 = wpool.tile([B, HC], F32, tag="r1", name=f"r1_{t}")
            nc.scalar.activation(r1[:, :], psR[:, HC:H], AF.Sigmoid)
            r_ = (r0, r1)

            # n = tanh(gx_n + r * pre_n), 256-col chunks
            n = wpool.tile([B, H], F32, tag="n", name=f"n_{t}")
            for c in range(2):
                cs = slice(c * HC, (c + 1) * HC)
                m = wpool.tile([B, HC], F32, tag=f"m{c}", name=f"m{c}_{t}")
                nc.vector.tensor_mul(m[:, :], r_[c][:, :], psN[:, cs])
                m2 = wpool.tile([B, HC], F32, tag=f"m2{c}", name=f"m2{c}_{t}")
                nc.vector.tensor_add(m2[:, :], m[:, :], psg[:, cs])
                nc.scalar.activation(n[:, cs], m2[:, :], AF.Tanh)

            # z = sigmoid(pre_z)
            z = wpool.tile([B, H], F32, tag="z", name=f"z_{t}")
            nc.scalar.activation(z[:, :], psZ[:, :], AF.Sigmoid)
            # h_new = n + z*(h - n)
            h_old = (ha, hb)
            new_h, new_hT = [], []
            tp_full = tpool.tile([128, NK * B], F32, tag="tp", name=f"tp_{t}", bufs=1)
            for c in range(2):
                tp = tp_full[:, 2 * c * B : 2 * (c + 1) * B]
                cs = slice(c * HC, (c + 1) * HC)
                d = wpool.tile([B, HC], F32, tag=f"d{c}", name=f"d{c}_{t}")
                nc.vector.tensor_sub(d[:, :], h_old[c][:, :], n[:, cs])
                e = wpool.tile([B, HC], F32, tag=f"e{c}", name=f"e{c}_{t}")
                nc.vector.tensor_mul(e[:, :], z[:, cs], d[:, :])
                hn = spool.tile(
                    [B, HC], F32, tag=("ha", "hb")[c], name=f"h{('a', 'b')[c]}_{t}"
                )
                nc.vector.tensor_add(hn[:, :], n[:, cs], e[:, :])
                for kk in range(2):
                    nc.tensor.transpose(
                        tp[:, kk * B : (kk + 1) * B],
                        hn[:, kk * 128 : (kk + 1) * 128],
                        ident64,
                    )
                hTn = spool.tile(
                    [128, 2 * B],
                    BF16,
                    tag=("hTa", "hTb")[c],
                    name=f"hT{('a', 'b')[c]}_{t}",
                )
                nc.vector.tensor_copy(hTn[:, :], tp[:, :])
                new_h.append(hn)
                new_hT.append(hTn)

            ha, hb = new_h
            hTa, hTb = new_hT

        # ---- head: out = relu(h) @ fc_w.T + fc_b ----
        reluh = wpool.tile([B, H], F32, tag="reluh", name="reluh")
        nc.scalar.activation(reluh[:, 0:HC], ha[:, :], AF.Relu)
        nc.scalar.activation(reluh[:, HC:H], hb[:, :], AF.Relu)
        tpf = tpool.tile([128, NK * B], F32, tag="tp", name="tp_fc", bufs=1)
        for k in range(NK):
            nc.tensor.transpose(
                tpf[:, k * B : (k + 1) * B], reluh[:, k * 128 : (k + 1) * 128], ident64
            )
        rhT = spool.tile([128, NK * B], F32, tag="rhT", name="rhT")
        nc.vector.tensor_copy(rhT[:, :], tpf[:, :])

        psf = gpool.tile([B, H], F32, tag="gx", name="ps_fc")
        nc.tensor.matmul(psf[:, 0:1], onesf[:, :], fcbf[0:1, 0:1], start=True, stop=False)
        for k in range(NK):
            nc.tensor.matmul(
                psf[:, 0:1],
                rhT[:, k * B : (k + 1) * B],
                fcwf[:, k : k + 1],
                start=False,
                stop=(k == NK - 1),
            )
        outsb = wpool.tile([B, 1], F32, tag="outsb", name="out_sb")
        nc.vector.tensor_copy(outsb[:, :], psf[:, 0:1])
        nc.sync.dma_start(out=out_d[:, :], in_=outsb[:, :])


_NC_CACHE: dict[int, bass.Bass] = {}


def _get_nc(T: int = T_FULL) -> bass.Bass:
    if T not in _NC_CACHE:
        _NC_CACHE[T] = build_nc(T)
    return _NC_CACHE[T]


def kernel(x, w_ih, w_hh, b_ih, b_hh, fc_w, fc_b, _trace=False, _tmpdir=None):
    x = np.ascontiguousarray(np.asarray(x, dtype=np.float32))
    nc = _get_nc(x.shape[1])
    shared = {
        "w_hh": np.ascontiguousarray(np.asarray(w_hh, np.float32)),
        "w_ih": np.ascontiguousarray(np.asarray(w_ih, np.float32)),
        "b_ih": np.ascontiguousarray(np.asarray(b_ih, np.float32)),
        "b_hh": np.ascontiguousarray(np.asarray(b_hh, np.float32)),
        "fc_w": np.ascontiguousarray(np.asarray(fc_w, np.float32)),
        "fc_b": np.ascontiguousarray(np.asarray(fc_b, np.float32)),
    }
    in_maps = [{"x": x[c * B : (c + 1) * B], **shared} for c in range(N_CORES)]
    res = run_bass_kernel_spmd(
        nc, in_maps, list(range(N_CORES)), trace=_trace, tmpdir=_tmpdir
    )
    out = np.concatenate([res.results[c]["out"] for c in range(N_CORES)], axis=0)
    if _trace:
        return out, res
    return out
